# revision 19
# baseline (speedup 1.0000x reference)
"""DeepWuKong GCN (3-layer GCNConv + max/mean pool + FFN) on 8 TRN2 NeuronCores.

v2 — fp16 on-chip, descriptor-lean gather:
  - 128 graphs -> 16/core, 512 node slots each (64 blocks of 128/core).
  - Table row numbering ROW(c,b,p) = c*8192 + p*64 + b so the node-major z
    table is ONE contiguous [128, 64, 128] SBUF->DRAM DMA per core; layers
    exchange it with a 2MB/rank fp16 AllGather.
  - Per-layer, per dst block: two dma_gather calls (int16 row idx, lo/hi
    halves of the 65536-row table) pull 256B fp16 rows edge-major; trailing
    -1 idx are descriptor-free padding; self-loops never gathered (injected
    as one diag(dis^2) matmul per block from the local z tile).
  - norm factoring: table rows hold dis[s]*z[s] (scaled during the ACT
    PSUM->SBUF copy); dis[d] is applied once per dst block post-aggregation
    (DVE mult with a static broadcast tile). The per-chunk one-hots are then
    pure 0/1 masks, identical across layers: host-precomputed fp16 tiles
    streamed from HBM via HWDGE (no on-chip one-hot builds). Self-loops are
    one identity matmul per block.
  - Aggregation: per 128-edge chunk, PE matmul msg^T @ mask into a
    feature-major PSUM block; DVE applies dis[d], ScalarE bias+ReLU.
  - Pooling/FFN identical to baseline (fp32).
"""
import sys

sys.path.insert(0, "/opt/trn_rl_repo")

import numpy as np

import concourse.bacc as bacc
import concourse.bass as bass
import concourse.mybir as mybir
import concourse.tile as tile
from concourse.bass_utils import run_bass_kernel_spmd

# ---- problem constants (hardcoded per spec) --------------------------------
N_NODES = 50000
N_EDGES = 600000
N_GRAPHS = 128
D = 128
N_LAYERS = 3
N_CORES = 8
GPC = N_GRAPHS // N_CORES      # 16 graphs per core
GSLOT = 512                    # node slots per graph (4 blocks of 128)
NLOC = GPC * GSLOT             # 8192 node slots per core
NBLK = NLOC // 128             # 64 blocks per core
BPG = GSLOT // 128             # blocks per graph
TOT = N_CORES * NLOC           # 65536 table rows
SPLIT = 32768                  # int16 gather index split

F32 = mybir.dt.float32
F16 = mybir.dt.float16
I16 = mybir.dt.int16

DMA_SCRATCH = 32768
N_QUEUES = 4
MAX_IDX_PER_CALL = 1024


# ===========================================================================
# host-side schedule construction
# ===========================================================================
def _build_schedule(x, edge_index, batch):
    x = np.asarray(x, np.float32)
    ei = np.asarray(edge_index).astype(np.int64)
    batch = np.asarray(batch).astype(np.int64)

    counts = np.bincount(batch, minlength=N_GRAPHS)
    assert counts.max() <= GSLOT, f"graph too big: {counts.max()}"

    deg = np.bincount(ei[1], minlength=N_NODES).astype(np.float64) + 1.0
    dis = 1.0 / np.sqrt(deg)

    graph_start = np.zeros(N_GRAPHS + 1, np.int64)
    np.cumsum(counts, out=graph_start[1:])

    # degree-balanced placement of each graph's nodes into its BPG blocks
    newslot = np.full(N_NODES, -1, np.int64)   # global slot: core*NLOC + b*128 + p
    for g in range(N_GRAPHS):
        nodes = np.arange(graph_start[g], graph_start[g + 1])
        if len(nodes) == 0:
            continue
        order = np.argsort(-deg[nodes], kind="stable")
        base = (g // GPC) * NLOC + (g % GPC) * GSLOT
        bin_load = np.zeros(BPG)
        bin_fill = np.zeros(BPG, np.int64)
        for n in nodes[order]:
            cand = np.argsort(bin_load, kind="stable")
            for b in cand:
                if bin_fill[b] < 128:
                    break
            newslot[n] = base + b * 128 + bin_fill[b]
            bin_fill[b] += 1
            bin_load[b] += deg[n]
    assert (newslot >= 0).all()

    core_of = newslot // NLOC
    lslot = newslot % NLOC
    blk_of = lslot // 128          # local block 0..63
    part_of = lslot % 128
    # table rows: 2 AllGather halves of 32 blocks; within a half the AG
    # concatenates ranks, each rank contributing [128 p, 32 b] node-major
    q_of = blk_of // 32
    row_of = q_of * SPLIT + core_of * 4096 + part_of * 32 + (blk_of % 32)

    # real edges only (self loops injected on device via diag matmul)
    src, dst = ei[0], ei[1]
    w = (dis[src] * dis[dst]).astype(np.float32)
    srow = row_of[src]
    dcore = core_of[dst]
    dblk = blk_of[dst]
    dpart = part_of[dst]
    hi = (srow >= SPLIT).astype(np.int64)

    # per (core, blk, bucket) counts -> shared K and Vmax across cores
    cnt = np.zeros((N_CORES, NBLK, 2), np.int64)
    np.add.at(cnt, (dcore, dblk, hi), 1)
    Vmax = cnt.max(axis=0)                      # [NBLK, 2]
    K = -(-Vmax // 128)                         # chunks (may be 0)
    assert (K * 128 <= MAX_IDX_PER_CALL).all(), f"call too big: {K.max()}"
    K_lo = K[:, 0].astype(int)
    K_hi = K[:, 1].astype(int)
    V_lo = Vmax[:, 0].astype(int)
    V_hi = Vmax[:, 1].astype(int)

    lo_off = np.zeros(NBLK + 1, np.int64)
    np.cumsum(K_lo * 128, out=lo_off[1:])
    hi_off = np.zeros(NBLK + 1, np.int64)
    np.cumsum(K_hi * 128, out=hi_off[1:])
    ch_off = np.zeros(NBLK + 1, np.int64)
    np.cumsum(K_lo + K_hi, out=ch_off[1:])
    nlo_slots = int(lo_off[-1])
    nhi_slots = int(hi_off[-1])
    NCH = int(ch_off[-1])

    idx_lo = np.full((N_CORES, max(nlo_slots, 16)), -1, np.int16)
    idx_hi = np.full((N_CORES, max(nhi_slots, 16)), -1, np.int16)

    # vectorized per-(core,blk,bucket) slot assignment
    sort = np.lexsort((hi, dblk, dcore))
    s_core, s_blk, s_hi = dcore[sort], dblk[sort], hi[sort]
    s_row, s_dp, s_w = srow[sort], dpart[sort], w[sort]
    gid = (s_core * NBLK + s_blk) * 2 + s_hi
    first = np.ones(len(gid), bool)
    first[1:] = gid[1:] != gid[:-1]
    gstart = np.zeros(len(gid), np.int64)
    idxs_first = np.flatnonzero(first)
    gstart[idxs_first] = idxs_first
    gstart = np.maximum.accumulate(gstart)
    pos = np.arange(len(gid)) - gstart

    slot = np.where(s_hi == 0, lo_off[s_blk], hi_off[s_blk]) + pos
    chcol = np.where(s_hi == 0, ch_off[s_blk], ch_off[s_blk] + K_lo[s_blk]) \
        + pos // 128
    val = np.where(s_hi == 0, s_row, s_row - SPLIT).astype(np.int16)
    lom = s_hi == 0
    idx_lo[s_core[lom], slot[lom]] = val[lom]
    idx_hi[s_core[~lom], slot[~lom]] = val[~lom]

    # filler: pad each (core,blk,bucket) from its own count up to Vmax with
    # idx=0 (real descriptor, zero one-hot) so num_idxs_reg is SPMD-uniform.
    for c in range(N_CORES):
        for b in range(NBLK):
            v = cnt[c, b, 0]
            if v < V_lo[b]:
                idx_lo[c, lo_off[b] + v: lo_off[b] + V_lo[b]] = 0
            v = cnt[c, b, 1]
            if v < V_hi[b]:
                idx_hi[c, hi_off[b] + v: hi_off[b] + V_hi[b]] = 0

    def wrap_idx(a):                 # [slots] -> [128, slots/16], 8x replicated
        pad = (-len(a)) % 16
        if pad:
            a = np.concatenate([a, np.full(pad, -1, np.int16)])
        w16 = a.reshape(-1, 16).T
        return np.tile(w16, (8, 1)).copy()

    idx_lo_w = np.stack([wrap_idx(idx_lo[c]) for c in range(N_CORES)])
    idx_hi_w = np.stack([wrap_idx(idx_hi[c]) for c in range(N_CORES)])

    # feature-major x per core, fp16, columns ordered by slot (b*128+p)
    xpad = np.zeros((N_CORES * NLOC, D), np.float32)
    xpad[newslot] = x
    x_fm = np.stack([xpad[c * NLOC:(c + 1) * NLOC].T.copy()
                     for c in range(N_CORES)]).astype(np.float16)

    # host-built mask tiles scaled by dis[d]: [core][128 edge, NCH*128]
    nch = max(NCH, 1)
    ohmask = np.zeros((N_CORES, 128, nch, 128), np.float16)
    cc, pp, hh = s_core, pos % 128, chcol
    ohmask[cc, pp, hh, s_dp] = dis[dst[sort]].astype(np.float16)
    ohmask = ohmask.reshape(N_CORES, 128, nch * 128)

    # per-slot dis, node-major: discol[c][p, b] = dis(node at (c,b,p))
    discol = np.zeros((N_CORES, 128, NBLK), np.float32)
    discol[core_of, part_of, blk_of] = dis.astype(np.float32)
    # self-loop diag tiles: diagdis[c][p, b*128+d] = (p==d) * dis(c,b,d)
    diagdis = np.zeros((N_CORES, 128, NBLK, 128), np.float16)
    diagdis[core_of, part_of, blk_of, part_of] = dis.astype(np.float16)
    diagdis = diagdis.reshape(N_CORES, 128, NBLK * 128)

    invcnt = (1.0 / np.maximum(counts, 1)).astype(np.float32)
    invcnt_rep = np.stack([
        np.tile(invcnt[c * GPC:(c + 1) * GPC], (128, 1)) for c in range(N_CORES)
    ]).astype(np.float32)

    return dict(
        K_lo=K_lo, K_hi=K_hi, V_lo=V_lo, V_hi=V_hi, NCH=NCH,
        nlo16=idx_lo_w.shape[2], nhi16=idx_hi_w.shape[2],
        lo_off=lo_off, hi_off=hi_off, ch_off=ch_off,
        idx_lo=idx_lo_w, idx_hi=idx_hi_w,
        ohmask=ohmask, discol=discol, diagdis=diagdis,
        x_fm=x_fm, invcnt_rep=invcnt_rep,
    )


# ===========================================================================
# device kernel
# ===========================================================================
def _build_kernel(sch):
    K_lo, K_hi = sch["K_lo"], sch["K_hi"]
    V_lo, V_hi = sch["V_lo"], sch["V_hi"]
    lo_off, hi_off, ch_off = sch["lo_off"], sch["hi_off"], sch["ch_off"]
    NCH = max(sch["NCH"], 1)
    NLO16 = sch["nlo16"]
    NHI16 = sch["nhi16"]

    # persist masks for the first PERS_B blocks in SBUF (budget ~112 chunks)
    PERS_B = 0
    cum = 0
    for b in range(NBLK):
        c = int(K_lo[b] + K_hi[b])
        if cum + c > 64:
            break
        cum += c
        PERS_B = b + 1

    nc = bacc.Bacc(
        "TRN2",
        target_bir_lowering=False,
        debug=False,
        num_devices=N_CORES,
        num_swdge_queues=N_QUEUES,
        dynamic_dma_scratch_size=DMA_SCRATCH,
    )

    xfm_d = nc.dram_tensor("xfm", [128, NLOC], F16, kind="ExternalInput")
    wc_d = nc.dram_tensor("wc", [N_LAYERS, 128, 128], F16, kind="ExternalInput")
    bct_d = nc.dram_tensor("bct", [128, N_LAYERS], F16, kind="ExternalInput")
    wffn_d = nc.dram_tensor("wffn", [256, 128], F32, kind="ExternalInput")
    bffnt_d = nc.dram_tensor("bffnt", [128, 1], F32, kind="ExternalInput")
    wfin_d = nc.dram_tensor("wfin", [128, 2], F32, kind="ExternalInput")
    bfinr_d = nc.dram_tensor("bfinr", [GPC, 2], F32, kind="ExternalInput")
    idxlo_d = nc.dram_tensor("idxlo", [128, NLO16], I16, kind="ExternalInput")
    idxhi_d = nc.dram_tensor("idxhi", [128, NHI16], I16, kind="ExternalInput")
    ohmask_d = nc.dram_tensor("ohmask", [128, NCH * 128], F16,
                              kind="ExternalInput")
    discol_d = nc.dram_tensor("discol", [128, NBLK], F32, kind="ExternalInput")
    diagdis_d = nc.dram_tensor("diagdis", [128, NBLK * 128], F16,
                               kind="ExternalInput")
    invc_d = nc.dram_tensor("invc", [128, GPC], F32, kind="ExternalInput")
    ident_d = nc.dram_tensor("ident", [128, 128], F16, kind="ExternalInput")
    out_d = nc.dram_tensor("out", [GPC, 2], F32, kind="ExternalOutput")


    RG = [list(range(N_CORES))]

    with tile.TileContext(nc) as tc:
        with (
            tc.tile_pool(name="consts", bufs=1) as consts,
            tc.tile_pool(name="hpool", bufs=2) as hpool,
            tc.tile_pool(name="zpool", bufs=2) as zpool,
            tc.tile_pool(name="gpool", bufs=12) as gpool,
            tc.tile_pool(name="gpool2", bufs=4) as gpool2,
            tc.tile_pool(name="ohpool", bufs=8) as ohpool,
            tc.tile_pool(name="spool", bufs=1) as spool,
            tc.tile_pool(name="ps128", bufs=2, space="PSUM") as ps128,
            tc.tile_pool(name="psagg", bufs=4, space="PSUM") as psagg,
            tc.tile_pool(name="psfin", bufs=1, space="PSUM") as psfin,
            tc.tile_pool(name="dram", bufs=1, space="DRAM") as dram,
        ):
            # ---- load constants -------------------------------------------
            wc_sb = consts.tile([128, N_LAYERS, 128], F16)
            nc.sync.dma_start(wc_sb[:], wc_d[:].rearrange("l p f -> p l f"))
            bct_sb = consts.tile([128, N_LAYERS], F16)
            nc.sync.dma_start(bct_sb[:], bct_d[:])
            wffn_sb = consts.tile([128, 2, 128], F32)
            nc.sync.dma_start(
                wffn_sb[:], wffn_d[:].rearrange("(h p) f -> p h f", p=128))
            bffnt_sb = consts.tile([128, 1], F32)
            nc.sync.dma_start(bffnt_sb[:], bffnt_d[:])
            wfin_sb = consts.tile([128, 2], F32)
            nc.sync.dma_start(wfin_sb[:], wfin_d[:])
            bfinr_sb = consts.tile([GPC, 2], F32)
            nc.sync.dma_start(bfinr_sb[:], bfinr_d[:])
            idxlo_sb = consts.tile([128, NLO16], I16)
            nc.sync.dma_start(idxlo_sb[:], idxlo_d[:])
            idxhi_sb = consts.tile([128, NHI16], I16)
            nc.sync.dma_start(idxhi_sb[:], idxhi_d[:])
            discol_sb = consts.tile([128, NBLK], F32)
            nc.sync.dma_start(discol_sb[:], discol_d[:])
            diagdis_sb = consts.tile([128, NBLK, 128], F16)
            nc.sync.dma_start(
                diagdis_sb[:], diagdis_d[:].rearrange("p (b d) -> p b d", d=128))
            invc_sb = consts.tile([128, GPC], F32)
            nc.sync.dma_start(invc_sb[:], invc_d[:])
            ident_sb = consts.tile([128, 128], F16)
            nc.sync.dma_start(ident_sb[:], ident_d[:])

            h_cur = hpool.tile([128, NLOC], F16, tag="h", name="h_init")
            nc.sync.dma_start(h_cur[:], xfm_d[:])

            # persisted mask tiles for the first PERS_B blocks (all layers)
            pers_ch = int(ch_off[PERS_B])
            mk_pers = None
            if pers_ch:
                mk_pers = consts.tile([128, pers_ch * 128], F16)
                nc.sync.dma_start(mk_pers[:], ohmask_d[:, 0:pers_ch * 128])

            def transform(l, h_src, z_nm):
                for b in range(NBLK):
                    transform_blk(l, h_src, z_nm, b)

            def transform_blk(l, h_src, z_nm, b):
                zps = ps128.tile([128, 128], F32, tag="zps",
                                 name=f"zps{l}_{b}")
                nc.tensor.matmul(
                    zps[:], h_src[:, b * 128:(b + 1) * 128],
                    wc_sb[:, l, :], start=True, stop=True)
                nc.scalar.activation(
                    z_nm[:, b, :], zps[:],
                    mybir.ActivationFunctionType.Copy,
                    scale=discol_sb[:, b:b + 1])

            def share_half(l, z_nm, q):
                z_own = dram.tile([128, 32, 128], F16, tag=f"zown{q}",
                                  bufs=2, name=f"zown{l}_{q}")
                nc.sync.dma_start(
                    z_own[:], z_nm[:, q * 32:(q + 1) * 32, :])
                z_half = dram.tile([SPLIT, 128], F16, tag=f"zfull{q}",
                                   bufs=2, addr_space="Shared",
                                   name=f"zfull{l}_{q}")
                nc.gpsimd.collective_compute(
                    "AllGather", mybir.AluOpType.bypass,
                    replica_groups=RG,
                    ins=[z_own[:].opt()],
                    outs=[z_half[:].opt()],
                )
                return z_half

            def share(l, z_nm):
                return [share_half(l, z_nm, 0), share_half(l, z_nm, 1)]

            # pre-zero all gather pool buffers: trailing -1 idx slots are
            # never written by DMA and virgin SBUF reads as fp16 NaN.
            kml = max(int(k) for k in K_lo) or 1
            kmh = max(int(k) for k in K_hi) or 1
            for i in range(12):
                gz = gpool.tile([128, kml, 128], F16, tag="glo",
                                name=f"gz_lo{i}")
                nc.vector.memset(gz[:], 0.0)
            for i in range(4):
                gz = gpool2.tile([128, kmh, 128], F16, tag="ghi",
                                 name=f"gz_hi{i}")
                nc.vector.memset(gz[:], 0.0)

            z_nm = zpool.tile([128, NBLK, 128], F16, tag="znm", name="znm0")
            transform(0, h_cur, z_nm)
            z_full = share(0, z_nm)


            for l in range(N_LAYERS):
                h_nxt = hpool.tile([128, NLOC], F16, tag="h", name=f"h{l + 1}")
                z_nm_nxt = None
                if l + 1 < N_LAYERS:
                    z_nm_nxt = zpool.tile([128, NBLK, 128], F16, tag="znm",
                                          name=f"znm{l + 1}")
                glo_pend = {}
                z_lo_nxt = None
                LAG = 10
                for step in range(NBLK + LAG):
                    if step < NBLK:
                        b = step
                        klo = int(K_lo[b])
                        if klo:
                            gt = gpool.tile([128, klo, 128], F16, tag="glo",
                                            name=f"glo{l}_{b}")
                            c0 = int(lo_off[b]) // 16
                            nc.gpsimd.dma_gather(
                                gt[:], z_full[0][:],
                                idxlo_sb[:, c0:c0 + klo * 8],
                                num_idxs=klo * 128, num_idxs_reg=int(V_lo[b]),
                                elem_size=128, queue_num=(2 * b) % N_QUEUES,
                            )
                            glo_pend[b] = gt
                    if step < LAG:
                        continue
                    b = step - LAG
                    klo, khi = int(K_lo[b]), int(K_hi[b])
                    vlo, vhi = int(V_lo[b]), int(V_hi[b])
                    glo = glo_pend.pop(b, None)
                    ghi = None
                    if khi:
                        ghi = gpool2.tile([128, khi, 128], F16, tag="ghi",
                                         name=f"ghi{l}_{b}")
                        c0 = int(hi_off[b]) // 16
                        nc.gpsimd.dma_gather(
                            ghi[:], z_full[1][:],
                            idxhi_sb[:, c0:c0 + khi * 8],
                            num_idxs=khi * 128, num_idxs_reg=vhi,
                            elem_size=128, queue_num=(2 * b + 1) % N_QUEUES,
                        )
                    ktot = klo + khi
                    ps = psagg.tile([128, 128], F32, tag="aggps",
                                    name=f"agg{l}_{b}")
                    # self-loop: dis_d^2 * z_d via diag(dis) on scaled table
                    nc.tensor.matmul(
                        ps[:], z_nm[:, b, :], diagdis_sb[:, b, :],
                        start=True, stop=(ktot == 0))
                    if ktot:
                        ch0 = int(ch_off[b])
                        if b < PERS_B:
                            mk = mk_pers[:, ch0 * 128:(ch0 + ktot) * 128]
                        else:
                            mkt = ohpool.tile([128, ktot * 128], F16,
                                              tag="oh", name=f"oh{l}_{b}")
                            nc.sync.dma_start(
                                mkt[:],
                                ohmask_d[:, ch0 * 128:(ch0 + ktot) * 128])
                            mk = mkt[:]
                        for j in range(ktot):
                            msg = glo[:, j, :] if j < klo \
                                else ghi[:, j - klo, :]
                            nc.tensor.matmul(
                                ps[:], msg, mk[:, j * 128:(j + 1) * 128],
                                start=False, stop=(j == ktot - 1))
                    nc.scalar.activation(
                        h_nxt[:, b * 128:(b + 1) * 128], ps[:],
                        mybir.ActivationFunctionType.Relu,
                        bias=bct_sb[:, l:l + 1])
                    if z_nm_nxt is not None:
                        transform_blk(l + 1, h_nxt, z_nm_nxt, b)
                        if b == 48:
                            z_lo_nxt = share_half(l + 1, z_nm_nxt, 0)
                if z_nm_nxt is not None:
                    z_nm = z_nm_nxt
                    z_full = [z_lo_nxt, share_half(l + 1, z_nm_nxt, 1)]
                h_cur = h_nxt


            # ---- pooling + FFN --------------------------------------------
            mx = spool.tile([128, GPC], F32)
            sm = spool.tile([128, GPC], F32)
            for g in range(GPC):
                nc.vector.tensor_reduce(
                    mx[:, g:g + 1], h_cur[:, g * GSLOT:(g + 1) * GSLOT],
                    mybir.AxisListType.X, mybir.AluOpType.max)
                nc.vector.tensor_reduce(
                    sm[:, g:g + 1], h_cur[:, g * GSLOT:(g + 1) * GSLOT],
                    mybir.AxisListType.X, mybir.AluOpType.add)
            mean = spool.tile([128, GPC], F32)
            nc.vector.tensor_tensor(
                mean[:], sm[:], invc_sb[:], mybir.AluOpType.mult)

            p1 = psfin.tile([128, GPC], F32, tag="p1")
            nc.tensor.matmul(p1[:], wffn_sb[:, 0, :], mx[:],
                             start=True, stop=False)
            nc.tensor.matmul(p1[:], wffn_sb[:, 1, :], mean[:],
                             start=False, stop=True)
            o1 = spool.tile([128, GPC], F32)
            nc.scalar.activation(
                o1[:], p1[:], mybir.ActivationFunctionType.Relu,
                bias=bffnt_sb[:, 0:1])

            p2 = psfin.tile([GPC, 2], F32, tag="p2")
            nc.tensor.matmul(p2[:], o1[:], wfin_sb[:], start=True, stop=True)
            osb = spool.tile([GPC, 2], F32)
            nc.vector.tensor_tensor(
                osb[:], p2[:], bfinr_sb[:], mybir.AluOpType.add)
            nc.sync.dma_start(out_d[:], osb[:])

    nc.compile()
    return nc


# ===========================================================================
# entry point
# ===========================================================================
_CACHE = {}


def kernel(x, Wc, bc, W_ffn, b_ffn, W_fin, b_fin, edge_index, batch):
    x = np.ascontiguousarray(np.asarray(x, np.float32))
    Wc = np.ascontiguousarray(np.asarray(Wc, np.float32))
    bc = np.ascontiguousarray(np.asarray(bc, np.float32))
    W_ffn = np.ascontiguousarray(np.asarray(W_ffn, np.float32))
    b_ffn = np.ascontiguousarray(np.asarray(b_ffn, np.float32))
    W_fin = np.ascontiguousarray(np.asarray(W_fin, np.float32))
    b_fin = np.ascontiguousarray(np.asarray(b_fin, np.float32))

    sch = _build_schedule(x, edge_index, batch)

    key = (sch["NCH"], sch["nlo16"], sch["nhi16"],
           tuple(sch["K_lo"]), tuple(sch["K_hi"]),
           tuple(sch["V_lo"]), tuple(sch["V_hi"]))
    if key not in _CACHE:
        _CACHE.clear()
        _CACHE[key] = _build_kernel(sch)
    nc = _CACHE[key]

    ident = np.eye(128, dtype=np.float16)
    bct = bc.T.astype(np.float16).copy()          # [128, 3]
    bffnt = b_ffn[:, None].copy()                 # [128, 1]
    bfinr = np.tile(b_fin[None, :], (GPC, 1)).astype(np.float32)

    in_maps = []
    for c in range(N_CORES):
        in_maps.append({
            "xfm": sch["x_fm"][c],
            "wc": Wc.astype(np.float16), "bct": bct,
            "wffn": W_ffn, "bffnt": bffnt,
            "wfin": W_fin, "bfinr": bfinr,
            "idxlo": sch["idx_lo"][c], "idxhi": sch["idx_hi"][c],
            "ohmask": sch["ohmask"][c], "discol": sch["discol"][c],
            "diagdis": sch["diagdis"][c],
            "invc": sch["invcnt_rep"][c],
            "ident": ident,
        })

    _CACHE["in_maps"] = in_maps
    res = run_bass_kernel_spmd(nc, in_maps, core_ids=list(range(N_CORES)))
    out = np.concatenate([res.results[c]["out"] for c in range(N_CORES)], 0)
    return out.astype(np.float32)


def timed_run(inputs=None):
    """Re-run the cached compiled kernel with profiling; returns exec ns."""
    import time
    nc = next(v for k, v in _CACHE.items() if k != "in_maps")
    in_maps = _CACHE["in_maps"]
    walls = []
    for _ in range(3):
        t0 = time.time()
        run_bass_kernel_spmd(nc, in_maps, core_ids=list(range(N_CORES)))
        walls.append(time.time() - t0)
    print(f"warm re-run walls: {[f'{w*1e3:.1f}ms' for w in walls]}")
    try:
        res = run_bass_kernel_spmd(
            nc, in_maps, core_ids=list(range(N_CORES)), trace=True)
        if res.exec_time_ns is not None:
            return res.exec_time_ns
    except Exception as e:
        print(f"(ntff profiling unavailable: {type(e).__name__}: {e}; "
              f"reporting warm wall-clock upper bound)")
    return int(min(walls) * 1e9)


if __name__ == "__main__":
    rng = np.random.default_rng(0)
    x = rng.standard_normal((N_NODES, D), dtype=np.float32)
    ei = rng.integers(0, N_NODES, (2, N_EDGES)).astype(np.int64)
    batch = np.sort(rng.integers(0, N_GRAPHS, N_NODES)).astype(np.int64)
    Wc = rng.standard_normal((3, D, D), dtype=np.float32) * 0.05
    out = kernel(x, Wc, np.zeros((3, D), np.float32),
                 rng.standard_normal((2 * D, D), dtype=np.float32) * 0.05,
                 np.zeros((D,), np.float32),
                 rng.standard_normal((D, 2), dtype=np.float32) * 0.05,
                 np.zeros((2,), np.float32), ei, batch)
    print(out.shape, out[:4])


# revision 20
# speedup vs baseline: 1.0333x; 1.0333x over previous
"""DeepWuKong GCN (3-layer GCNConv + max/mean pool + FFN) on 8 TRN2 NeuronCores.

v2 — fp16 on-chip, descriptor-lean gather:
  - 128 graphs -> 16/core, 512 node slots each (64 blocks of 128/core).
  - Table row numbering ROW(c,b,p) = c*8192 + p*64 + b so the node-major z
    table is ONE contiguous [128, 64, 128] SBUF->DRAM DMA per core; layers
    exchange it with a 2MB/rank fp16 AllGather.
  - Per-layer, per dst block: two dma_gather calls (int16 row idx, lo/hi
    halves of the 65536-row table) pull 256B fp16 rows edge-major; trailing
    -1 idx are descriptor-free padding; self-loops never gathered (injected
    as one diag(dis^2) matmul per block from the local z tile).
  - norm factoring: table rows hold dis[s]*z[s] (scaled during the ACT
    PSUM->SBUF copy); dis[d] is applied once per dst block post-aggregation
    (DVE mult with a static broadcast tile). The per-chunk one-hots are then
    pure 0/1 masks, identical across layers: host-precomputed fp16 tiles
    streamed from HBM via HWDGE (no on-chip one-hot builds). Self-loops are
    one identity matmul per block.
  - Aggregation: per 128-edge chunk, PE matmul msg^T @ mask into a
    feature-major PSUM block; DVE applies dis[d], ScalarE bias+ReLU.
  - Pooling/FFN identical to baseline (fp32).
"""
import sys

sys.path.insert(0, "/opt/trn_rl_repo")

import numpy as np

import concourse.bacc as bacc
import concourse.bass as bass
import concourse.mybir as mybir
import concourse.tile as tile
from concourse.bass_utils import run_bass_kernel_spmd

# ---- problem constants (hardcoded per spec) --------------------------------
N_NODES = 50000
N_EDGES = 600000
N_GRAPHS = 128
D = 128
N_LAYERS = 3
N_CORES = 8
GPC = N_GRAPHS // N_CORES      # 16 graphs per core
GSLOT = 512                    # node slots per graph (4 blocks of 128)
NLOC = GPC * GSLOT             # 8192 node slots per core
NBLK = NLOC // 128             # 64 blocks per core
BPG = GSLOT // 128             # blocks per graph
TOT = N_CORES * NLOC           # 65536 table rows
SPLIT = 32768                  # int16 gather index split

F32 = mybir.dt.float32
F16 = mybir.dt.float16
I16 = mybir.dt.int16

DMA_SCRATCH = 32768
N_QUEUES = 4
MAX_IDX_PER_CALL = 1024


# ===========================================================================
# host-side schedule construction
# ===========================================================================
def _build_schedule(x, edge_index, batch):
    x = np.asarray(x, np.float32)
    ei = np.asarray(edge_index).astype(np.int64)
    batch = np.asarray(batch).astype(np.int64)

    counts = np.bincount(batch, minlength=N_GRAPHS)
    assert counts.max() <= GSLOT, f"graph too big: {counts.max()}"

    deg = np.bincount(ei[1], minlength=N_NODES).astype(np.float64) + 1.0
    dis = 1.0 / np.sqrt(deg)

    graph_start = np.zeros(N_GRAPHS + 1, np.int64)
    np.cumsum(counts, out=graph_start[1:])

    # degree-balanced placement of each graph's nodes into its BPG blocks
    newslot = np.full(N_NODES, -1, np.int64)   # global slot: core*NLOC + b*128 + p
    for g in range(N_GRAPHS):
        nodes = np.arange(graph_start[g], graph_start[g + 1])
        if len(nodes) == 0:
            continue
        order = np.argsort(-deg[nodes], kind="stable")
        base = (g // GPC) * NLOC + (g % GPC) * GSLOT
        bin_load = np.zeros(BPG)
        bin_fill = np.zeros(BPG, np.int64)
        for n in nodes[order]:
            cand = np.argsort(bin_load, kind="stable")
            for b in cand:
                if bin_fill[b] < 128:
                    break
            newslot[n] = base + b * 128 + bin_fill[b]
            bin_fill[b] += 1
            bin_load[b] += deg[n]
    assert (newslot >= 0).all()

    core_of = newslot // NLOC
    lslot = newslot % NLOC
    blk_of = lslot // 128          # local block 0..63
    part_of = lslot % 128
    # table rows: 2 AllGather halves of 32 blocks; within a half the AG
    # concatenates ranks, each rank contributing [128 p, 32 b] node-major
    q_of = blk_of // 32
    row_of = q_of * SPLIT + core_of * 4096 + part_of * 32 + (blk_of % 32)

    # real edges only (self loops injected on device via diag matmul)
    src, dst = ei[0], ei[1]
    w = (dis[src] * dis[dst]).astype(np.float32)
    srow = row_of[src]
    dcore = core_of[dst]
    dblk = blk_of[dst]
    dpart = part_of[dst]
    hi = (srow >= SPLIT).astype(np.int64)

    # per (core, blk, bucket) counts -> shared K and Vmax across cores
    cnt = np.zeros((N_CORES, NBLK, 2), np.int64)
    np.add.at(cnt, (dcore, dblk, hi), 1)
    Vmax = cnt.max(axis=0)                      # [NBLK, 2]
    K = -(-Vmax // 128)                         # chunks (may be 0)
    assert (K * 128 <= MAX_IDX_PER_CALL).all(), f"call too big: {K.max()}"
    K_lo = K[:, 0].astype(int)
    K_hi = K[:, 1].astype(int)
    V_lo = Vmax[:, 0].astype(int)
    V_hi = Vmax[:, 1].astype(int)

    lo_off = np.zeros(NBLK + 1, np.int64)
    np.cumsum(K_lo * 128, out=lo_off[1:])
    hi_off = np.zeros(NBLK + 1, np.int64)
    np.cumsum(K_hi * 128, out=hi_off[1:])
    ch_off = np.zeros(NBLK + 1, np.int64)
    np.cumsum(K_lo + K_hi, out=ch_off[1:])
    nlo_slots = int(lo_off[-1])
    nhi_slots = int(hi_off[-1])
    NCH = int(ch_off[-1])

    idx_lo = np.full((N_CORES, max(nlo_slots, 16)), -1, np.int16)
    idx_hi = np.full((N_CORES, max(nhi_slots, 16)), -1, np.int16)

    # vectorized per-(core,blk,bucket) slot assignment
    sort = np.lexsort((hi, dblk, dcore))
    s_core, s_blk, s_hi = dcore[sort], dblk[sort], hi[sort]
    s_row, s_dp, s_w = srow[sort], dpart[sort], w[sort]
    gid = (s_core * NBLK + s_blk) * 2 + s_hi
    first = np.ones(len(gid), bool)
    first[1:] = gid[1:] != gid[:-1]
    gstart = np.zeros(len(gid), np.int64)
    idxs_first = np.flatnonzero(first)
    gstart[idxs_first] = idxs_first
    gstart = np.maximum.accumulate(gstart)
    pos = np.arange(len(gid)) - gstart

    slot = np.where(s_hi == 0, lo_off[s_blk], hi_off[s_blk]) + pos
    chcol = np.where(s_hi == 0, ch_off[s_blk], ch_off[s_blk] + K_lo[s_blk]) \
        + pos // 128
    val = np.where(s_hi == 0, s_row, s_row - SPLIT).astype(np.int16)
    lom = s_hi == 0
    idx_lo[s_core[lom], slot[lom]] = val[lom]
    idx_hi[s_core[~lom], slot[~lom]] = val[~lom]

    # filler: pad each (core,blk,bucket) from its own count up to Vmax with
    # idx=0 (real descriptor, zero one-hot) so num_idxs_reg is SPMD-uniform.
    for c in range(N_CORES):
        for b in range(NBLK):
            v = cnt[c, b, 0]
            if v < V_lo[b]:
                idx_lo[c, lo_off[b] + v: lo_off[b] + V_lo[b]] = 0
            v = cnt[c, b, 1]
            if v < V_hi[b]:
                idx_hi[c, hi_off[b] + v: hi_off[b] + V_hi[b]] = 0

    def wrap_idx(a):                 # [slots] -> [128, slots/16], 8x replicated
        pad = (-len(a)) % 16
        if pad:
            a = np.concatenate([a, np.full(pad, -1, np.int16)])
        w16 = a.reshape(-1, 16).T
        return np.tile(w16, (8, 1)).copy()

    idx_lo_w = np.stack([wrap_idx(idx_lo[c]) for c in range(N_CORES)])
    idx_hi_w = np.stack([wrap_idx(idx_hi[c]) for c in range(N_CORES)])

    # feature-major x per core, fp16, columns ordered by slot (b*128+p)
    xpad = np.zeros((N_CORES * NLOC, D), np.float32)
    xpad[newslot] = x
    x_fm = np.stack([xpad[c * NLOC:(c + 1) * NLOC].T.copy()
                     for c in range(N_CORES)]).astype(np.float16)

    # host-built mask tiles scaled by dis[d]: [core][128 edge, NCH*128]
    nch = max(NCH, 1)
    ohmask = np.zeros((N_CORES, 128, nch, 128), np.float16)
    cc, pp, hh = s_core, pos % 128, chcol
    ohmask[cc, pp, hh, s_dp] = dis[dst[sort]].astype(np.float16)
    ohmask = ohmask.reshape(N_CORES, 128, nch * 128)

    # per-slot dis, node-major: discol[c][p, b] = dis(node at (c,b,p))
    discol = np.zeros((N_CORES, 128, NBLK), np.float32)
    discol[core_of, part_of, blk_of] = dis.astype(np.float32)
    # self-loop diag tiles: diagdis[c][p, b*128+d] = (p==d) * dis(c,b,d)
    diagdis = np.zeros((N_CORES, 128, NBLK, 128), np.float16)
    diagdis[core_of, part_of, blk_of, part_of] = dis.astype(np.float16)
    diagdis = diagdis.reshape(N_CORES, 128, NBLK * 128)

    invcnt = (1.0 / np.maximum(counts, 1)).astype(np.float32)
    invcnt_rep = np.stack([
        np.tile(invcnt[c * GPC:(c + 1) * GPC], (128, 1)) for c in range(N_CORES)
    ]).astype(np.float32)

    return dict(
        K_lo=K_lo, K_hi=K_hi, V_lo=V_lo, V_hi=V_hi, NCH=NCH,
        nlo16=idx_lo_w.shape[2], nhi16=idx_hi_w.shape[2],
        lo_off=lo_off, hi_off=hi_off, ch_off=ch_off,
        idx_lo=idx_lo_w, idx_hi=idx_hi_w,
        ohmask=ohmask, discol=discol, diagdis=diagdis,
        x_fm=x_fm, invcnt_rep=invcnt_rep,
    )


# ===========================================================================
# device kernel
# ===========================================================================
def _build_kernel(sch):
    K_lo, K_hi = sch["K_lo"], sch["K_hi"]
    V_lo, V_hi = sch["V_lo"], sch["V_hi"]
    lo_off, hi_off, ch_off = sch["lo_off"], sch["hi_off"], sch["ch_off"]
    NCH = max(sch["NCH"], 1)
    NLO16 = sch["nlo16"]
    NHI16 = sch["nhi16"]

    # persist masks for the first PERS_B blocks in SBUF (budget ~112 chunks)
    PERS_B = 0
    cum = 0
    for b in range(NBLK):
        c = int(K_lo[b] + K_hi[b])
        if cum + c > 64:
            break
        cum += c
        PERS_B = b + 1

    nc = bacc.Bacc(
        "TRN2",
        target_bir_lowering=False,
        debug=False,
        num_devices=N_CORES,
        num_swdge_queues=N_QUEUES,
        dynamic_dma_scratch_size=DMA_SCRATCH,
    )

    xfm_d = nc.dram_tensor("xfm", [128, NLOC], F16, kind="ExternalInput")
    wc_d = nc.dram_tensor("wc", [N_LAYERS, 128, 128], F16, kind="ExternalInput")
    bct_d = nc.dram_tensor("bct", [128, N_LAYERS], F16, kind="ExternalInput")
    wffn_d = nc.dram_tensor("wffn", [256, 128], F32, kind="ExternalInput")
    bffnt_d = nc.dram_tensor("bffnt", [128, 1], F32, kind="ExternalInput")
    wfin_d = nc.dram_tensor("wfin", [128, 2], F32, kind="ExternalInput")
    bfinr_d = nc.dram_tensor("bfinr", [GPC, 2], F32, kind="ExternalInput")
    idxlo_d = nc.dram_tensor("idxlo", [128, NLO16], I16, kind="ExternalInput")
    idxhi_d = nc.dram_tensor("idxhi", [128, NHI16], I16, kind="ExternalInput")
    ohmask_d = nc.dram_tensor("ohmask", [128, NCH * 128], F16,
                              kind="ExternalInput")
    discol_d = nc.dram_tensor("discol", [128, NBLK], F32, kind="ExternalInput")
    diagdis_d = nc.dram_tensor("diagdis", [128, NBLK * 128], F16,
                               kind="ExternalInput")
    invc_d = nc.dram_tensor("invc", [128, GPC], F32, kind="ExternalInput")
    ident_d = nc.dram_tensor("ident", [128, 128], F16, kind="ExternalInput")
    out_d = nc.dram_tensor("out", [GPC, 2], F32, kind="ExternalOutput")


    RG = [list(range(N_CORES))]

    with tile.TileContext(nc) as tc:
        with (
            tc.tile_pool(name="consts", bufs=1) as consts,
            tc.tile_pool(name="hpool", bufs=2) as hpool,
            tc.tile_pool(name="zpool", bufs=2) as zpool,
            tc.tile_pool(name="gpool", bufs=12) as gpool,
            tc.tile_pool(name="gpool2", bufs=4) as gpool2,
            tc.tile_pool(name="ohpool", bufs=8) as ohpool,
            tc.tile_pool(name="spool", bufs=1) as spool,
            tc.tile_pool(name="ps128", bufs=2, space="PSUM") as ps128,
            tc.tile_pool(name="psagg", bufs=4, space="PSUM") as psagg,
            tc.tile_pool(name="psfin", bufs=1, space="PSUM") as psfin,
            tc.tile_pool(name="dram", bufs=1, space="DRAM") as dram,
        ):
            # ---- load constants -------------------------------------------
            wc_sb = consts.tile([128, N_LAYERS, 128], F16)
            nc.sync.dma_start(wc_sb[:], wc_d[:].rearrange("l p f -> p l f"))
            bct_sb = consts.tile([128, N_LAYERS], F16)
            nc.sync.dma_start(bct_sb[:], bct_d[:])
            wffn_sb = consts.tile([128, 2, 128], F32)
            nc.sync.dma_start(
                wffn_sb[:], wffn_d[:].rearrange("(h p) f -> p h f", p=128))
            bffnt_sb = consts.tile([128, 1], F32)
            nc.sync.dma_start(bffnt_sb[:], bffnt_d[:])
            wfin_sb = consts.tile([128, 2], F32)
            nc.sync.dma_start(wfin_sb[:], wfin_d[:])
            bfinr_sb = consts.tile([GPC, 2], F32)
            nc.sync.dma_start(bfinr_sb[:], bfinr_d[:])
            idxlo_sb = consts.tile([128, NLO16], I16)
            nc.sync.dma_start(idxlo_sb[:], idxlo_d[:])
            idxhi_sb = consts.tile([128, NHI16], I16)
            nc.sync.dma_start(idxhi_sb[:], idxhi_d[:])
            discol_sb = consts.tile([128, NBLK], F32)
            nc.sync.dma_start(discol_sb[:], discol_d[:])
            diagdis_sb = consts.tile([128, NBLK, 128], F16)
            nc.sync.dma_start(
                diagdis_sb[:], diagdis_d[:].rearrange("p (b d) -> p b d", d=128))
            invc_sb = consts.tile([128, GPC], F32)
            nc.sync.dma_start(invc_sb[:], invc_d[:])
            ident_sb = consts.tile([128, 128], F16)
            nc.sync.dma_start(ident_sb[:], ident_d[:])

            h_cur = hpool.tile([128, NLOC], F16, tag="h", name="h_init")
            nc.sync.dma_start(h_cur[:], xfm_d[:])

            # persisted mask tiles for the first PERS_B blocks (all layers)
            pers_ch = int(ch_off[PERS_B])
            mk_pers = None
            if pers_ch:
                mk_pers = consts.tile([128, pers_ch * 128], F16)
                nc.sync.dma_start(mk_pers[:], ohmask_d[:, 0:pers_ch * 128])

            def transform(l, h_src, z_nm):
                for b in range(NBLK):
                    transform_blk(l, h_src, z_nm, b)

            def transform_blk(l, h_src, z_nm, b):
                zps = ps128.tile([128, 128], F32, tag="zps",
                                 name=f"zps{l}_{b}")
                nc.tensor.matmul(
                    zps[:], h_src[:, b * 128:(b + 1) * 128],
                    wc_sb[:, l, :], start=True, stop=True)
                nc.scalar.activation(
                    z_nm[:, b, :], zps[:],
                    mybir.ActivationFunctionType.Copy,
                    scale=discol_sb[:, b:b + 1])

            def share_half(l, z_nm, q):
                z_own = dram.tile([128, 32, 128], F16, tag=f"zown{q}",
                                  bufs=2, name=f"zown{l}_{q}")
                nc.sync.dma_start(
                    z_own[:], z_nm[:, q * 32:(q + 1) * 32, :])
                z_half = dram.tile([SPLIT, 128], F16, tag=f"zfull{q}",
                                   bufs=2, addr_space="Shared",
                                   name=f"zfull{l}_{q}")
                nc.gpsimd.collective_compute(
                    "AllGather", mybir.AluOpType.bypass,
                    replica_groups=RG,
                    ins=[z_own[:].opt()],
                    outs=[z_half[:].opt()],
                )
                return z_half

            def share(l, z_nm):
                return [share_half(l, z_nm, 0), share_half(l, z_nm, 1)]

            # pre-zero all gather pool buffers: trailing -1 idx slots are
            # never written by DMA and virgin SBUF reads as fp16 NaN.
            kml = max(int(k) for k in K_lo) or 1
            kmh = max(int(k) for k in K_hi) or 1
            for i in range(12):
                gz = gpool.tile([128, kml, 128], F16, tag="glo",
                                name=f"gz_lo{i}")
                nc.vector.memset(gz[:], 0.0)
            for i in range(4):
                gz = gpool2.tile([128, kmh, 128], F16, tag="ghi",
                                 name=f"gz_hi{i}")
                nc.vector.memset(gz[:], 0.0)

            z_nm = zpool.tile([128, NBLK, 128], F16, tag="znm", name="znm0")
            transform(0, h_cur, z_nm)
            z_full = share(0, z_nm)


            for l in range(N_LAYERS):
                h_nxt = hpool.tile([128, NLOC], F16, tag="h", name=f"h{l + 1}")
                z_nm_nxt = None
                if l + 1 < N_LAYERS:
                    z_nm_nxt = zpool.tile([128, NBLK, 128], F16, tag="znm",
                                          name=f"znm{l + 1}")
                glo_pend = {}
                LAG = 10
                for step in range(NBLK + LAG):
                    if step < NBLK:
                        b = step
                        klo = int(K_lo[b])
                        if klo:
                            gt = gpool.tile([128, klo, 128], F16, tag="glo",
                                            name=f"glo{l}_{b}")
                            c0 = int(lo_off[b]) // 16
                            nc.gpsimd.dma_gather(
                                gt[:], z_full[0][:],
                                idxlo_sb[:, c0:c0 + klo * 8],
                                num_idxs=klo * 128, num_idxs_reg=int(V_lo[b]),
                                elem_size=128, queue_num=(2 * b) % N_QUEUES,
                            )
                            glo_pend[b] = gt
                    if step < LAG:
                        continue
                    b = step - LAG
                    klo, khi = int(K_lo[b]), int(K_hi[b])
                    vlo, vhi = int(V_lo[b]), int(V_hi[b])
                    glo = glo_pend.pop(b, None)
                    ghi = None
                    if khi:
                        ghi = gpool2.tile([128, khi, 128], F16, tag="ghi",
                                         name=f"ghi{l}_{b}")
                        c0 = int(hi_off[b]) // 16
                        nc.gpsimd.dma_gather(
                            ghi[:], z_full[1][:],
                            idxhi_sb[:, c0:c0 + khi * 8],
                            num_idxs=khi * 128, num_idxs_reg=vhi,
                            elem_size=128, queue_num=(2 * b + 1) % N_QUEUES,
                        )
                    ktot = klo + khi
                    ps = psagg.tile([128, 128], F32, tag="aggps",
                                    name=f"agg{l}_{b}")
                    # self-loop: dis_d^2 * z_d via diag(dis) on scaled table
                    nc.tensor.matmul(
                        ps[:], z_nm[:, b, :], diagdis_sb[:, b, :],
                        start=True, stop=(ktot == 0))
                    if ktot:
                        ch0 = int(ch_off[b])
                        if b < PERS_B:
                            mk = mk_pers[:, ch0 * 128:(ch0 + ktot) * 128]
                        else:
                            mkt = ohpool.tile([128, ktot * 128], F16,
                                              tag="oh", name=f"oh{l}_{b}")
                            nc.sync.dma_start(
                                mkt[:],
                                ohmask_d[:, ch0 * 128:(ch0 + ktot) * 128])
                            mk = mkt[:]
                        for j in range(ktot):
                            msg = glo[:, j, :] if j < klo \
                                else ghi[:, j - klo, :]
                            nc.tensor.matmul(
                                ps[:], msg, mk[:, j * 128:(j + 1) * 128],
                                start=False, stop=(j == ktot - 1))
                    nc.scalar.activation(
                        h_nxt[:, b * 128:(b + 1) * 128], ps[:],
                        mybir.ActivationFunctionType.Relu,
                        bias=bct_sb[:, l:l + 1])
                    if z_nm_nxt is not None:
                        transform_blk(l + 1, h_nxt, z_nm_nxt, b)
                if z_nm_nxt is not None:
                    z_nm = z_nm_nxt
                    z_full = share(l + 1, z_nm_nxt)
                h_cur = h_nxt


            # ---- pooling + FFN --------------------------------------------
            mx = spool.tile([128, GPC], F32)
            sm = spool.tile([128, GPC], F32)
            for g in range(GPC):
                nc.vector.tensor_reduce(
                    mx[:, g:g + 1], h_cur[:, g * GSLOT:(g + 1) * GSLOT],
                    mybir.AxisListType.X, mybir.AluOpType.max)
                nc.vector.tensor_reduce(
                    sm[:, g:g + 1], h_cur[:, g * GSLOT:(g + 1) * GSLOT],
                    mybir.AxisListType.X, mybir.AluOpType.add)
            mean = spool.tile([128, GPC], F32)
            nc.vector.tensor_tensor(
                mean[:], sm[:], invc_sb[:], mybir.AluOpType.mult)

            p1 = psfin.tile([128, GPC], F32, tag="p1")
            nc.tensor.matmul(p1[:], wffn_sb[:, 0, :], mx[:],
                             start=True, stop=False)
            nc.tensor.matmul(p1[:], wffn_sb[:, 1, :], mean[:],
                             start=False, stop=True)
            o1 = spool.tile([128, GPC], F32)
            nc.scalar.activation(
                o1[:], p1[:], mybir.ActivationFunctionType.Relu,
                bias=bffnt_sb[:, 0:1])

            p2 = psfin.tile([GPC, 2], F32, tag="p2")
            nc.tensor.matmul(p2[:], o1[:], wfin_sb[:], start=True, stop=True)
            osb = spool.tile([GPC, 2], F32)
            nc.vector.tensor_tensor(
                osb[:], p2[:], bfinr_sb[:], mybir.AluOpType.add)
            nc.sync.dma_start(out_d[:], osb[:])

    nc.compile()
    return nc


# ===========================================================================
# entry point
# ===========================================================================
_CACHE = {}


def kernel(x, Wc, bc, W_ffn, b_ffn, W_fin, b_fin, edge_index, batch):
    x = np.ascontiguousarray(np.asarray(x, np.float32))
    Wc = np.ascontiguousarray(np.asarray(Wc, np.float32))
    bc = np.ascontiguousarray(np.asarray(bc, np.float32))
    W_ffn = np.ascontiguousarray(np.asarray(W_ffn, np.float32))
    b_ffn = np.ascontiguousarray(np.asarray(b_ffn, np.float32))
    W_fin = np.ascontiguousarray(np.asarray(W_fin, np.float32))
    b_fin = np.ascontiguousarray(np.asarray(b_fin, np.float32))

    sch = _build_schedule(x, edge_index, batch)

    key = (sch["NCH"], sch["nlo16"], sch["nhi16"],
           tuple(sch["K_lo"]), tuple(sch["K_hi"]),
           tuple(sch["V_lo"]), tuple(sch["V_hi"]))
    if key not in _CACHE:
        _CACHE.clear()
        _CACHE[key] = _build_kernel(sch)
    nc = _CACHE[key]

    ident = np.eye(128, dtype=np.float16)
    bct = bc.T.astype(np.float16).copy()          # [128, 3]
    bffnt = b_ffn[:, None].copy()                 # [128, 1]
    bfinr = np.tile(b_fin[None, :], (GPC, 1)).astype(np.float32)

    in_maps = []
    for c in range(N_CORES):
        in_maps.append({
            "xfm": sch["x_fm"][c],
            "wc": Wc.astype(np.float16), "bct": bct,
            "wffn": W_ffn, "bffnt": bffnt,
            "wfin": W_fin, "bfinr": bfinr,
            "idxlo": sch["idx_lo"][c], "idxhi": sch["idx_hi"][c],
            "ohmask": sch["ohmask"][c], "discol": sch["discol"][c],
            "diagdis": sch["diagdis"][c],
            "invc": sch["invcnt_rep"][c],
            "ident": ident,
        })

    _CACHE["in_maps"] = in_maps
    res = run_bass_kernel_spmd(nc, in_maps, core_ids=list(range(N_CORES)))
    out = np.concatenate([res.results[c]["out"] for c in range(N_CORES)], 0)
    return out.astype(np.float32)


def timed_run(inputs=None):
    """Re-run the cached compiled kernel with profiling; returns exec ns."""
    import time
    nc = next(v for k, v in _CACHE.items() if k != "in_maps")
    in_maps = _CACHE["in_maps"]
    walls = []
    for _ in range(3):
        t0 = time.time()
        run_bass_kernel_spmd(nc, in_maps, core_ids=list(range(N_CORES)))
        walls.append(time.time() - t0)
    print(f"warm re-run walls: {[f'{w*1e3:.1f}ms' for w in walls]}")
    try:
        res = run_bass_kernel_spmd(
            nc, in_maps, core_ids=list(range(N_CORES)), trace=True)
        if res.exec_time_ns is not None:
            return res.exec_time_ns
    except Exception as e:
        print(f"(ntff profiling unavailable: {type(e).__name__}: {e}; "
              f"reporting warm wall-clock upper bound)")
    return int(min(walls) * 1e9)


if __name__ == "__main__":
    rng = np.random.default_rng(0)
    x = rng.standard_normal((N_NODES, D), dtype=np.float32)
    ei = rng.integers(0, N_NODES, (2, N_EDGES)).astype(np.int64)
    batch = np.sort(rng.integers(0, N_GRAPHS, N_NODES)).astype(np.int64)
    Wc = rng.standard_normal((3, D, D), dtype=np.float32) * 0.05
    out = kernel(x, Wc, np.zeros((3, D), np.float32),
                 rng.standard_normal((2 * D, D), dtype=np.float32) * 0.05,
                 np.zeros((D,), np.float32),
                 rng.standard_normal((D, 2), dtype=np.float32) * 0.05,
                 np.zeros((2,), np.float32), ei, batch)
    print(out.shape, out[:4])


# revision 22
# speedup vs baseline: 1.0339x; 1.0006x over previous
"""DeepWuKong GCN (3-layer GCNConv + max/mean pool + FFN) on 8 TRN2 NeuronCores.

v2 — fp16 on-chip, descriptor-lean gather:
  - 128 graphs -> 16/core, 512 node slots each (64 blocks of 128/core).
  - Table row numbering ROW(c,b,p) = c*8192 + p*64 + b so the node-major z
    table is ONE contiguous [128, 64, 128] SBUF->DRAM DMA per core; layers
    exchange it with a 2MB/rank fp16 AllGather.
  - Per-layer, per dst block: two dma_gather calls (int16 row idx, lo/hi
    halves of the 65536-row table) pull 256B fp16 rows edge-major; trailing
    -1 idx are descriptor-free padding; self-loops never gathered (injected
    as one diag(dis^2) matmul per block from the local z tile).
  - norm factoring: table rows hold dis[s]*z[s] (scaled during the ACT
    PSUM->SBUF copy); dis[d] is applied once per dst block post-aggregation
    (DVE mult with a static broadcast tile). The per-chunk one-hots are then
    pure 0/1 masks, identical across layers: host-precomputed fp16 tiles
    streamed from HBM via HWDGE (no on-chip one-hot builds). Self-loops are
    one identity matmul per block.
  - Aggregation: per 128-edge chunk, PE matmul msg^T @ mask into a
    feature-major PSUM block; DVE applies dis[d], ScalarE bias+ReLU.
  - Pooling/FFN identical to baseline (fp32).
"""
import sys

sys.path.insert(0, "/opt/trn_rl_repo")

import numpy as np

import concourse.bacc as bacc
import concourse.bass as bass
import concourse.mybir as mybir
import concourse.tile as tile
from concourse.bass_utils import run_bass_kernel_spmd

# ---- problem constants (hardcoded per spec) --------------------------------
N_NODES = 50000
N_EDGES = 600000
N_GRAPHS = 128
D = 128
N_LAYERS = 3
N_CORES = 8
GPC = N_GRAPHS // N_CORES      # 16 graphs per core
GSLOT = 512                    # node slots per graph (4 blocks of 128)
NLOC = GPC * GSLOT             # 8192 node slots per core
NBLK = NLOC // 128             # 64 blocks per core
BPG = GSLOT // 128             # blocks per graph
TOT = N_CORES * NLOC           # 65536 table rows
SPLIT = 32768                  # int16 gather index split

F32 = mybir.dt.float32
F16 = mybir.dt.float16
I16 = mybir.dt.int16

DMA_SCRATCH = 32768
N_QUEUES = 4
MAX_IDX_PER_CALL = 1024


# ===========================================================================
# host-side schedule construction
# ===========================================================================
def _build_schedule(x, edge_index, batch):
    x = np.asarray(x, np.float32)
    ei = np.asarray(edge_index).astype(np.int64)
    batch = np.asarray(batch).astype(np.int64)

    counts = np.bincount(batch, minlength=N_GRAPHS)
    assert counts.max() <= GSLOT, f"graph too big: {counts.max()}"

    deg = np.bincount(ei[1], minlength=N_NODES).astype(np.float64) + 1.0
    dis = 1.0 / np.sqrt(deg)

    graph_start = np.zeros(N_GRAPHS + 1, np.int64)
    np.cumsum(counts, out=graph_start[1:])

    # degree-balanced placement of each graph's nodes into its BPG blocks
    newslot = np.full(N_NODES, -1, np.int64)   # global slot: core*NLOC + b*128 + p
    for g in range(N_GRAPHS):
        nodes = np.arange(graph_start[g], graph_start[g + 1])
        if len(nodes) == 0:
            continue
        order = np.argsort(-deg[nodes], kind="stable")
        base = (g // GPC) * NLOC + (g % GPC) * GSLOT
        bin_load = np.zeros(BPG)
        bin_fill = np.zeros(BPG, np.int64)
        for n in nodes[order]:
            cand = np.argsort(bin_load, kind="stable")
            for b in cand:
                if bin_fill[b] < 128:
                    break
            newslot[n] = base + b * 128 + bin_fill[b]
            bin_fill[b] += 1
            bin_load[b] += deg[n]
    assert (newslot >= 0).all()

    core_of = newslot // NLOC
    lslot = newslot % NLOC
    blk_of = lslot // 128          # local block 0..63
    part_of = lslot % 128
    # table rows: 2 AllGather halves of 32 blocks; within a half the AG
    # concatenates ranks, each rank contributing [128 p, 32 b] node-major
    q_of = blk_of // 32
    row_of = q_of * SPLIT + core_of * 4096 + part_of * 32 + (blk_of % 32)

    # real edges only (self loops injected on device via diag matmul)
    src, dst = ei[0], ei[1]
    w = (dis[src] * dis[dst]).astype(np.float32)
    srow = row_of[src]
    dcore = core_of[dst]
    dblk = blk_of[dst]
    dpart = part_of[dst]
    hi = (srow >= SPLIT).astype(np.int64)

    # per (core, blk, bucket) counts -> shared K and Vmax across cores
    cnt = np.zeros((N_CORES, NBLK, 2), np.int64)
    np.add.at(cnt, (dcore, dblk, hi), 1)
    Vmax = cnt.max(axis=0)                      # [NBLK, 2]
    K = -(-Vmax // 128)                         # chunks (may be 0)
    assert (K * 128 <= MAX_IDX_PER_CALL).all(), f"call too big: {K.max()}"
    K_lo = K[:, 0].astype(int)
    K_hi = K[:, 1].astype(int)
    V_lo = Vmax[:, 0].astype(int)
    V_hi = Vmax[:, 1].astype(int)

    lo_off = np.zeros(NBLK + 1, np.int64)
    np.cumsum(K_lo * 128, out=lo_off[1:])
    hi_off = np.zeros(NBLK + 1, np.int64)
    np.cumsum(K_hi * 128, out=hi_off[1:])
    ch_off = np.zeros(NBLK + 1, np.int64)
    np.cumsum(K_lo + K_hi, out=ch_off[1:])
    nlo_slots = int(lo_off[-1])
    nhi_slots = int(hi_off[-1])
    NCH = int(ch_off[-1])

    idx_lo = np.full((N_CORES, max(nlo_slots, 16)), -1, np.int16)
    idx_hi = np.full((N_CORES, max(nhi_slots, 16)), -1, np.int16)

    # vectorized per-(core,blk,bucket) slot assignment
    sort = np.lexsort((hi, dblk, dcore))
    s_core, s_blk, s_hi = dcore[sort], dblk[sort], hi[sort]
    s_row, s_dp, s_w = srow[sort], dpart[sort], w[sort]
    gid = (s_core * NBLK + s_blk) * 2 + s_hi
    first = np.ones(len(gid), bool)
    first[1:] = gid[1:] != gid[:-1]
    gstart = np.zeros(len(gid), np.int64)
    idxs_first = np.flatnonzero(first)
    gstart[idxs_first] = idxs_first
    gstart = np.maximum.accumulate(gstart)
    pos = np.arange(len(gid)) - gstart

    slot = np.where(s_hi == 0, lo_off[s_blk], hi_off[s_blk]) + pos
    chcol = np.where(s_hi == 0, ch_off[s_blk], ch_off[s_blk] + K_lo[s_blk]) \
        + pos // 128
    val = np.where(s_hi == 0, s_row, s_row - SPLIT).astype(np.int16)
    lom = s_hi == 0
    idx_lo[s_core[lom], slot[lom]] = val[lom]
    idx_hi[s_core[~lom], slot[~lom]] = val[~lom]

    # filler: pad each (core,blk,bucket) from its own count up to Vmax with
    # idx=0 (real descriptor, zero one-hot) so num_idxs_reg is SPMD-uniform.
    for c in range(N_CORES):
        for b in range(NBLK):
            v = cnt[c, b, 0]
            if v < V_lo[b]:
                idx_lo[c, lo_off[b] + v: lo_off[b] + V_lo[b]] = 0
            v = cnt[c, b, 1]
            if v < V_hi[b]:
                idx_hi[c, hi_off[b] + v: hi_off[b] + V_hi[b]] = 0

    def wrap_idx(a):                 # [slots] -> [128, slots/16], 8x replicated
        pad = (-len(a)) % 16
        if pad:
            a = np.concatenate([a, np.full(pad, -1, np.int16)])
        w16 = a.reshape(-1, 16).T
        return np.tile(w16, (8, 1)).copy()

    idx_lo_w = np.stack([wrap_idx(idx_lo[c]) for c in range(N_CORES)])
    idx_hi_w = np.stack([wrap_idx(idx_hi[c]) for c in range(N_CORES)])

    # feature-major x per core, fp16, columns ordered by slot (b*128+p)
    xpad = np.zeros((N_CORES * NLOC, D), np.float32)
    xpad[newslot] = x
    x_fm = np.stack([xpad[c * NLOC:(c + 1) * NLOC].T.copy()
                     for c in range(N_CORES)]).astype(np.float16)

    # host-built mask tiles scaled by dis[d]: [core][128 edge, NCH*128]
    nch = max(NCH, 1)
    ohmask = np.zeros((N_CORES, 128, nch, 128), np.float16)
    cc, pp, hh = s_core, pos % 128, chcol
    ohmask[cc, pp, hh, s_dp] = dis[dst[sort]].astype(np.float16)
    ohmask = ohmask.reshape(N_CORES, 128, nch * 128)

    # per-slot dis, node-major: discol[c][p, b] = dis(node at (c,b,p))
    discol = np.zeros((N_CORES, 128, NBLK), np.float32)
    discol[core_of, part_of, blk_of] = dis.astype(np.float32)
    # self-loop diag tiles: diagdis[c][p, b*128+d] = (p==d) * dis(c,b,d)
    diagdis = np.zeros((N_CORES, 128, NBLK, 128), np.float16)
    diagdis[core_of, part_of, blk_of, part_of] = dis.astype(np.float16)
    diagdis = diagdis.reshape(N_CORES, 128, NBLK * 128)

    invcnt = (1.0 / np.maximum(counts, 1)).astype(np.float32)
    invcnt_rep = np.stack([
        np.tile(invcnt[c * GPC:(c + 1) * GPC], (128, 1)) for c in range(N_CORES)
    ]).astype(np.float32)

    return dict(
        K_lo=K_lo, K_hi=K_hi, V_lo=V_lo, V_hi=V_hi, NCH=NCH,
        nlo16=idx_lo_w.shape[2], nhi16=idx_hi_w.shape[2],
        lo_off=lo_off, hi_off=hi_off, ch_off=ch_off,
        idx_lo=idx_lo_w, idx_hi=idx_hi_w,
        ohmask=ohmask, discol=discol, diagdis=diagdis,
        x_fm=x_fm, invcnt_rep=invcnt_rep,
    )


# ===========================================================================
# device kernel
# ===========================================================================
def _build_kernel(sch):
    K_lo, K_hi = sch["K_lo"], sch["K_hi"]
    V_lo, V_hi = sch["V_lo"], sch["V_hi"]
    lo_off, hi_off, ch_off = sch["lo_off"], sch["hi_off"], sch["ch_off"]
    NCH = max(sch["NCH"], 1)
    NLO16 = sch["nlo16"]
    NHI16 = sch["nhi16"]

    # persist masks for the first PERS_B blocks in SBUF (budget ~112 chunks)
    PERS_B = 0
    cum = 0
    for b in range(NBLK):
        c = int(K_lo[b] + K_hi[b])
        if cum + c > 128:
            break
        cum += c
        PERS_B = b + 1

    nc = bacc.Bacc(
        "TRN2",
        target_bir_lowering=False,
        debug=False,
        num_devices=N_CORES,
        num_swdge_queues=N_QUEUES,
        dynamic_dma_scratch_size=DMA_SCRATCH,
    )

    xfm_d = nc.dram_tensor("xfm", [128, NLOC], F16, kind="ExternalInput")
    wc_d = nc.dram_tensor("wc", [N_LAYERS, 128, 128], F16, kind="ExternalInput")
    bct_d = nc.dram_tensor("bct", [128, N_LAYERS], F16, kind="ExternalInput")
    wffn_d = nc.dram_tensor("wffn", [256, 128], F32, kind="ExternalInput")
    bffnt_d = nc.dram_tensor("bffnt", [128, 1], F32, kind="ExternalInput")
    wfin_d = nc.dram_tensor("wfin", [128, 2], F32, kind="ExternalInput")
    bfinr_d = nc.dram_tensor("bfinr", [GPC, 2], F32, kind="ExternalInput")
    idxlo_d = nc.dram_tensor("idxlo", [128, NLO16], I16, kind="ExternalInput")
    idxhi_d = nc.dram_tensor("idxhi", [128, NHI16], I16, kind="ExternalInput")
    ohmask_d = nc.dram_tensor("ohmask", [128, NCH * 128], F16,
                              kind="ExternalInput")
    discol_d = nc.dram_tensor("discol", [128, NBLK], F32, kind="ExternalInput")
    diagdis_d = nc.dram_tensor("diagdis", [128, NBLK * 128], F16,
                               kind="ExternalInput")
    invc_d = nc.dram_tensor("invc", [128, GPC], F32, kind="ExternalInput")
    ident_d = nc.dram_tensor("ident", [128, 128], F16, kind="ExternalInput")
    out_d = nc.dram_tensor("out", [GPC, 2], F32, kind="ExternalOutput")


    RG = [list(range(N_CORES))]

    with tile.TileContext(nc) as tc:
        with (
            tc.tile_pool(name="consts", bufs=1) as consts,
            tc.tile_pool(name="hpool", bufs=2) as hpool,
            tc.tile_pool(name="zpool", bufs=2) as zpool,
            tc.tile_pool(name="gpool", bufs=16) as gpool,
            tc.tile_pool(name="gpool2", bufs=4) as gpool2,
            tc.tile_pool(name="ohpool", bufs=8) as ohpool,
            tc.tile_pool(name="spool", bufs=1) as spool,
            tc.tile_pool(name="ps128", bufs=2, space="PSUM") as ps128,
            tc.tile_pool(name="psagg", bufs=4, space="PSUM") as psagg,
            tc.tile_pool(name="psfin", bufs=1, space="PSUM") as psfin,
            tc.tile_pool(name="dram", bufs=1, space="DRAM") as dram,
        ):
            # ---- load constants (critical-path inputs first) --------------
            h_cur0 = hpool.tile([128, NLOC], F16, tag="h", name="h_init")
            nc.sync.dma_start(h_cur0[:], xfm_d[:])
            wc_sb = consts.tile([128, N_LAYERS, 128], F16)
            nc.sync.dma_start(wc_sb[:], wc_d[:].rearrange("l p f -> p l f"))
            discol_sb = consts.tile([128, NBLK], F32)
            nc.sync.dma_start(discol_sb[:], discol_d[:])
            idxlo_sb = consts.tile([128, NLO16], I16)
            nc.sync.dma_start(idxlo_sb[:], idxlo_d[:])
            idxhi_sb = consts.tile([128, NHI16], I16)
            nc.sync.dma_start(idxhi_sb[:], idxhi_d[:])
            diagdis_sb = consts.tile([128, NBLK, 128], F16)
            nc.sync.dma_start(
                diagdis_sb[:], diagdis_d[:].rearrange("p (b d) -> p b d", d=128))
            bct_sb = consts.tile([128, N_LAYERS], F16)
            nc.sync.dma_start(bct_sb[:], bct_d[:])
            wffn_sb = consts.tile([128, 2, 128], F32)
            nc.sync.dma_start(
                wffn_sb[:], wffn_d[:].rearrange("(h p) f -> p h f", p=128))
            bffnt_sb = consts.tile([128, 1], F32)
            nc.sync.dma_start(bffnt_sb[:], bffnt_d[:])
            wfin_sb = consts.tile([128, 2], F32)
            nc.sync.dma_start(wfin_sb[:], wfin_d[:])
            bfinr_sb = consts.tile([GPC, 2], F32)
            nc.sync.dma_start(bfinr_sb[:], bfinr_d[:])
            invc_sb = consts.tile([128, GPC], F32)
            nc.sync.dma_start(invc_sb[:], invc_d[:])
            ident_sb = consts.tile([128, 128], F16)
            nc.sync.dma_start(ident_sb[:], ident_d[:])

            # persisted mask tiles for the first PERS_B blocks (all layers)
            pers_ch = int(ch_off[PERS_B])
            mk_pers = None
            if pers_ch:
                mk_pers = consts.tile([128, pers_ch * 128], F16)
                nc.sync.dma_start(mk_pers[:], ohmask_d[:, 0:pers_ch * 128])

            def transform(l, h_src, z_nm):
                for b in range(NBLK):
                    transform_blk(l, h_src, z_nm, b)

            def transform_blk(l, h_src, z_nm, b):
                zps = ps128.tile([128, 128], F32, tag="zps",
                                 name=f"zps{l}_{b}")
                nc.tensor.matmul(
                    zps[:], h_src[:, b * 128:(b + 1) * 128],
                    wc_sb[:, l, :], start=True, stop=True)
                nc.scalar.activation(
                    z_nm[:, b, :], zps[:],
                    mybir.ActivationFunctionType.Copy,
                    scale=discol_sb[:, b:b + 1])

            def share_half(l, z_nm, q):
                z_own = dram.tile([128, 32, 128], F16, tag=f"zown{q}",
                                  bufs=2, name=f"zown{l}_{q}")
                nc.sync.dma_start(
                    z_own[:], z_nm[:, q * 32:(q + 1) * 32, :])
                z_half = dram.tile([SPLIT, 128], F16, tag=f"zfull{q}",
                                   bufs=2, addr_space="Shared",
                                   name=f"zfull{l}_{q}")
                nc.gpsimd.collective_compute(
                    "AllGather", mybir.AluOpType.bypass,
                    replica_groups=RG,
                    ins=[z_own[:].opt()],
                    outs=[z_half[:].opt()],
                )
                return z_half

            def share(l, z_nm):
                return [share_half(l, z_nm, 0), share_half(l, z_nm, 1)]

            # pre-zero all gather pool buffers: trailing -1 idx slots are
            # never written by DMA and virgin SBUF reads as fp16 NaN.
            kml = max(int(k) for k in K_lo) or 1
            kmh = max(int(k) for k in K_hi) or 1
            for i in range(16):
                gz = gpool.tile([128, kml, 128], F16, tag="glo",
                                name=f"gz_lo{i}")
                nc.vector.memset(gz[:], 0.0)
            for i in range(4):
                gz = gpool2.tile([128, kmh, 128], F16, tag="ghi",
                                 name=f"gz_hi{i}")
                nc.vector.memset(gz[:], 0.0)

            h_cur = h_cur0
            z_nm = zpool.tile([128, NBLK, 128], F16, tag="znm", name="znm0")
            transform(0, h_cur, z_nm)
            z_full = share(0, z_nm)


            for l in range(N_LAYERS):
                h_nxt = hpool.tile([128, NLOC], F16, tag="h", name=f"h{l + 1}")
                z_nm_nxt = None
                if l + 1 < N_LAYERS:
                    z_nm_nxt = zpool.tile([128, NBLK, 128], F16, tag="znm",
                                          name=f"znm{l + 1}")
                glo_pend = {}
                z_lo_nxt = None
                LAG = 14
                for step in range(NBLK + LAG):
                    if step < NBLK:
                        b = step
                        klo = int(K_lo[b])
                        if klo:
                            gt = gpool.tile([128, klo, 128], F16, tag="glo",
                                            name=f"glo{l}_{b}")
                            c0 = int(lo_off[b]) // 16
                            nc.gpsimd.dma_gather(
                                gt[:], z_full[0][:],
                                idxlo_sb[:, c0:c0 + klo * 8],
                                num_idxs=klo * 128, num_idxs_reg=int(V_lo[b]),
                                elem_size=128, queue_num=(2 * b) % N_QUEUES,
                            )
                            glo_pend[b] = gt
                    if step < LAG:
                        continue
                    b = step - LAG
                    klo, khi = int(K_lo[b]), int(K_hi[b])
                    vlo, vhi = int(V_lo[b]), int(V_hi[b])
                    glo = glo_pend.pop(b, None)
                    ghi = None
                    if khi:
                        ghi = gpool2.tile([128, khi, 128], F16, tag="ghi",
                                         name=f"ghi{l}_{b}")
                        c0 = int(hi_off[b]) // 16
                        nc.gpsimd.dma_gather(
                            ghi[:], z_full[1][:],
                            idxhi_sb[:, c0:c0 + khi * 8],
                            num_idxs=khi * 128, num_idxs_reg=vhi,
                            elem_size=128, queue_num=(2 * b + 1) % N_QUEUES,
                        )
                    ktot = klo + khi
                    ps = psagg.tile([128, 128], F32, tag="aggps",
                                    name=f"agg{l}_{b}")
                    # self-loop: dis_d^2 * z_d via diag(dis) on scaled table
                    nc.tensor.matmul(
                        ps[:], z_nm[:, b, :], diagdis_sb[:, b, :],
                        start=True, stop=(ktot == 0))
                    if ktot:
                        ch0 = int(ch_off[b])
                        if b < PERS_B:
                            mk = mk_pers[:, ch0 * 128:(ch0 + ktot) * 128]
                        else:
                            mkt = ohpool.tile([128, ktot * 128], F16,
                                              tag="oh", name=f"oh{l}_{b}")
                            nc.sync.dma_start(
                                mkt[:],
                                ohmask_d[:, ch0 * 128:(ch0 + ktot) * 128])
                            mk = mkt[:]
                        for j in range(ktot):
                            msg = glo[:, j, :] if j < klo \
                                else ghi[:, j - klo, :]
                            nc.tensor.matmul(
                                ps[:], msg, mk[:, j * 128:(j + 1) * 128],
                                start=False, stop=(j == ktot - 1))
                    nc.scalar.activation(
                        h_nxt[:, b * 128:(b + 1) * 128], ps[:],
                        mybir.ActivationFunctionType.Relu,
                        bias=bct_sb[:, l:l + 1])
                    if z_nm_nxt is not None:
                        transform_blk(l + 1, h_nxt, z_nm_nxt, b)
                        if b == 56:
                            z_lo_nxt = share_half(l + 1, z_nm_nxt, 0)
                if z_nm_nxt is not None:
                    z_nm = z_nm_nxt
                    z_full = [z_lo_nxt, share_half(l + 1, z_nm_nxt, 1)]
                h_cur = h_nxt


            # ---- pooling + FFN --------------------------------------------
            mx = spool.tile([128, GPC], F32)
            sm = spool.tile([128, GPC], F32)
            for g in range(GPC):
                nc.vector.tensor_reduce(
                    mx[:, g:g + 1], h_cur[:, g * GSLOT:(g + 1) * GSLOT],
                    mybir.AxisListType.X, mybir.AluOpType.max)
                nc.vector.tensor_reduce(
                    sm[:, g:g + 1], h_cur[:, g * GSLOT:(g + 1) * GSLOT],
                    mybir.AxisListType.X, mybir.AluOpType.add)
            mean = spool.tile([128, GPC], F32)
            nc.vector.tensor_tensor(
                mean[:], sm[:], invc_sb[:], mybir.AluOpType.mult)

            p1 = psfin.tile([128, GPC], F32, tag="p1")
            nc.tensor.matmul(p1[:], wffn_sb[:, 0, :], mx[:],
                             start=True, stop=False)
            nc.tensor.matmul(p1[:], wffn_sb[:, 1, :], mean[:],
                             start=False, stop=True)
            o1 = spool.tile([128, GPC], F32)
            nc.scalar.activation(
                o1[:], p1[:], mybir.ActivationFunctionType.Relu,
                bias=bffnt_sb[:, 0:1])

            p2 = psfin.tile([GPC, 2], F32, tag="p2")
            nc.tensor.matmul(p2[:], o1[:], wfin_sb[:], start=True, stop=True)
            osb = spool.tile([GPC, 2], F32)
            nc.vector.tensor_tensor(
                osb[:], p2[:], bfinr_sb[:], mybir.AluOpType.add)
            nc.sync.dma_start(out_d[:], osb[:])

    nc.compile()
    return nc


# ===========================================================================
# entry point
# ===========================================================================
_CACHE = {}


def kernel(x, Wc, bc, W_ffn, b_ffn, W_fin, b_fin, edge_index, batch):
    x = np.ascontiguousarray(np.asarray(x, np.float32))
    Wc = np.ascontiguousarray(np.asarray(Wc, np.float32))
    bc = np.ascontiguousarray(np.asarray(bc, np.float32))
    W_ffn = np.ascontiguousarray(np.asarray(W_ffn, np.float32))
    b_ffn = np.ascontiguousarray(np.asarray(b_ffn, np.float32))
    W_fin = np.ascontiguousarray(np.asarray(W_fin, np.float32))
    b_fin = np.ascontiguousarray(np.asarray(b_fin, np.float32))

    sch = _build_schedule(x, edge_index, batch)

    key = (sch["NCH"], sch["nlo16"], sch["nhi16"],
           tuple(sch["K_lo"]), tuple(sch["K_hi"]),
           tuple(sch["V_lo"]), tuple(sch["V_hi"]))
    if key not in _CACHE:
        _CACHE.clear()
        _CACHE[key] = _build_kernel(sch)
    nc = _CACHE[key]

    ident = np.eye(128, dtype=np.float16)
    bct = bc.T.astype(np.float16).copy()          # [128, 3]
    bffnt = b_ffn[:, None].copy()                 # [128, 1]
    bfinr = np.tile(b_fin[None, :], (GPC, 1)).astype(np.float32)

    in_maps = []
    for c in range(N_CORES):
        in_maps.append({
            "xfm": sch["x_fm"][c],
            "wc": Wc.astype(np.float16), "bct": bct,
            "wffn": W_ffn, "bffnt": bffnt,
            "wfin": W_fin, "bfinr": bfinr,
            "idxlo": sch["idx_lo"][c], "idxhi": sch["idx_hi"][c],
            "ohmask": sch["ohmask"][c], "discol": sch["discol"][c],
            "diagdis": sch["diagdis"][c],
            "invc": sch["invcnt_rep"][c],
            "ident": ident,
        })

    _CACHE["in_maps"] = in_maps
    res = run_bass_kernel_spmd(nc, in_maps, core_ids=list(range(N_CORES)))
    out = np.concatenate([res.results[c]["out"] for c in range(N_CORES)], 0)
    return out.astype(np.float32)


def timed_run(inputs=None):
    """Re-run the cached compiled kernel with profiling; returns exec ns."""
    import time
    nc = next(v for k, v in _CACHE.items() if k != "in_maps")
    in_maps = _CACHE["in_maps"]
    walls = []
    for _ in range(3):
        t0 = time.time()
        run_bass_kernel_spmd(nc, in_maps, core_ids=list(range(N_CORES)))
        walls.append(time.time() - t0)
    print(f"warm re-run walls: {[f'{w*1e3:.1f}ms' for w in walls]}")
    try:
        res = run_bass_kernel_spmd(
            nc, in_maps, core_ids=list(range(N_CORES)), trace=True)
        if res.exec_time_ns is not None:
            return res.exec_time_ns
    except Exception as e:
        print(f"(ntff profiling unavailable: {type(e).__name__}: {e}; "
              f"reporting warm wall-clock upper bound)")
    return int(min(walls) * 1e9)


if __name__ == "__main__":
    rng = np.random.default_rng(0)
    x = rng.standard_normal((N_NODES, D), dtype=np.float32)
    ei = rng.integers(0, N_NODES, (2, N_EDGES)).astype(np.int64)
    batch = np.sort(rng.integers(0, N_GRAPHS, N_NODES)).astype(np.int64)
    Wc = rng.standard_normal((3, D, D), dtype=np.float32) * 0.05
    out = kernel(x, Wc, np.zeros((3, D), np.float32),
                 rng.standard_normal((2 * D, D), dtype=np.float32) * 0.05,
                 np.zeros((D,), np.float32),
                 rng.standard_normal((D, 2), dtype=np.float32) * 0.05,
                 np.zeros((2,), np.float32), ei, batch)
    print(out.shape, out[:4])
